# revision 1
# baseline (speedup 1.0000x reference)
"""Complex dot-product attention on 8 Trainium2 NeuronCores.

Reference computation (per batch b):
    sr = (qr @ kr^T - qi @ ki^T) / sqrt(D)      si = (qr @ ki^T + qi @ kr^T) / sqrt(D)
    ar = softmax(sr, axis=k)                    ai = softmax(si, axis=k)
    out_r = ar @ vr - ai @ vi                   out_i = ar @ vi + ai @ vr

Shapes: q/k/v [B=4, S=4096, D=64, 2] fp32, interleaved (real, imag) last dim.

Sharding: data-parallel over batch x sequence-parallel over query rows.
Core c handles batch b = c//2, query rows [h*2048, (h+1)*2048) with h = c%2,
and all 4096 keys of that batch (K/V replicated per batch pair). No
collectives; the host slices inputs per core and concatenates outputs.

Per-core kernel math trick: with Q, K, V kept in their NATURAL interleaved
layout ([s, 2d] where col 2d = real_d, col 2d+1 = imag_d):
    sr[q,k] = sum_{2d} Qneg[q,:] * K[k,:]   with Qneg = [qr0, -qi0, qr1, -qi1, ...]
    si[q,k] = sum_{2d} Qswap[q,:] * K[k,:]  with Qswap = [qi0, qr0, qi1, qr1, ...]
so both score components contract over the full 128-wide interleaved axis
against the SAME natural K. Scores are computed TRANSPOSED ([k, q]) so that
the attention matmul (contraction over k) can consume the exp'd scores
directly from SBUF as the moving operand:
    P_a[m, q] = sum_k V[k, m]  * Er[k, q]   (V natural as stationary)
    P_b[m, q] = sum_k V2[k, m] * Ei[k, q]   (V2 = [-vi0, vr0, -vi1, vr1, ...])
    out_T[m, q] = P_a[m,q] / sum_r[q] + P_b[m,q] / sum_i[q]
which lands rows m = (d, complex)-interleaved, exactly the HBM layout after a
final 128x128 PE transpose. Softmax skips max-subtraction (scores are
O(+-6) for randn inputs; exp stays comfortably inside fp32 range).
"""

import os

import numpy as np

import concourse.bass as bass
import concourse.mybir as mybir
import concourse.tile as tile
from concourse import bacc

F32 = mybir.dt.float32
F32R = mybir.dt.float32r
BF16 = mybir.dt.bfloat16
EXP = mybir.ActivationFunctionType.Exp
MULT = mybir.AluOpType.mult
ADD = mybir.AluOpType.add

B, S, D = 4, 4096, 64
W = 2 * D  # 128 interleaved columns
NCORES = 8
SQ = B * S // NCORES  # 2048 query rows per core
SCALE = 1.0 / float(np.sqrt(D))


def build_nc(sq=SQ, sk=S, gk=2, qb_size=512):
    """Build the per-core SPMD bass program."""
    nq = sq // 128   # q 128-row chunks
    nk = sk // 128   # k tiles
    nqb = sq // qb_size
    njb = qb_size // 128
    ngroups = nk // gk
    assert ngroups % 2 == 0

    nc = bacc.Bacc(target_bir_lowering=False)

    q_d = nc.declare_dram_parameter("q", [sq, W], F32, isOutput=False)
    k_d = nc.declare_dram_parameter("k", [sk, W], F32, isOutput=False)
    v_d = nc.declare_dram_parameter("v", [sk, W], F32, isOutput=False)
    ident_d = nc.declare_dram_parameter("ident", [128, 128], F32, isOutput=False)
    swapneg_d = nc.declare_dram_parameter("swapneg", [128, 128], F32R, isOutput=False)
    onesm_d = nc.declare_dram_parameter("onesm", [128, 128], BF16, isOutput=False)
    sign_d = nc.declare_dram_parameter("sign", [128, 1], F32, isOutput=False)
    out_d = nc.declare_dram_parameter("out", [sq, W], F32, isOutput=True)

    qv = q_d.rearrange("(c p) n -> p c n", p=128)  # [128, nq, 128]
    kv = k_d.rearrange("(c p) n -> p c n", p=128)
    vv = v_d.rearrange("(c p) n -> p c n", p=128)
    # out row = a*qb_size + j*128 + p
    ov = out_d.rearrange("(a j p) n -> a p j n", p=128, j=njb)

    with tile.TileContext(nc) as tc:
        with (
            tc.tile_pool(name="const", bufs=1) as constp,
            tc.tile_pool(name="big", bufs=1) as big,
            tc.tile_pool(name="epool", bufs=3) as epool,
            tc.tile_pool(name="small", bufs=2) as small,
            # PSUM budget: 8 banks of [128 x 512 fp32].
            tc.tile_pool(name="psA", bufs=2, space=bass.MemorySpace.PSUM) as psA,  # scores: 2x2 banks
            tc.tile_pool(name="psB", bufs=2, space=bass.MemorySpace.PSUM) as psB,  # AV accum: 2x1
            tc.tile_pool(name="psC", bufs=2, space=bass.MemorySpace.PSUM) as psC,  # sums + out-tr: 2x1
        ):
            CH = 4  # tiles per DMA chunk
            # sync queue: ident first (transposes need it), then K chunks.
            # scalar (2nd HWDGE queue): Q chunks + remaining consts, concurrently.
            # Small consts lead both queues so nothing downstream waits on
            # them; K chunks alternate between the two HWDGE queues.
            ident = constp.tile([128, 128], F32, tag="ident")
            nc.sync.dma_start(ident[:], ident_d[:])
            sign = constp.tile([128, 1], F32, tag="sign")
            nc.scalar.dma_start(sign[:], sign_d[:])
            swapneg = constp.tile([128, 128], F32R, tag="swapneg")
            nc.scalar.dma_start(swapneg[:], swapneg_d[:])
            onesm = constp.tile([128, 128], BF16, tag="onesm")
            nc.scalar.dma_start(onesm[:], onesm_d[:])
            kchunks, qchunks = [], []
            for i, c0 in enumerate(range(0, nk, CH)):
                t = big.tile([128, min(CH, nk - c0), 128], F32, tag=f"knat{c0}")
                eng = nc.sync if i % 2 == 0 else nc.scalar
                eng.dma_start(t[:], kv[:, c0:c0 + t.shape[1], :])
                kchunks.append(t)
            for c0 in range(0, nq, CH):
                t = big.tile([128, min(CH, nq - c0), 128], F32, tag=f"qnat{c0}")
                nc.scalar.dma_start(t[:], qv[:, c0:c0 + t.shape[1], :])
                qchunks.append(t)

            # K^T: [2d, k]
            kT = big.tile([128, sk], F32R, tag="kT")
            for c in range(nk):
                ps = psA.tile([128, 128], F32, tag="sc")
                nc.tensor.transpose(ps[:], kchunks[c // CH][:, c % CH, :], ident[:])
                nc.vector.tensor_copy(kT[:, c * 128:(c + 1) * 128], ps[:])

            # Qneg^T: [2d, q] with odd partitions negated (sign = +1/-1 per partition)
            qnegT = big.tile([128, sq], F32R, tag="qnegT")
            for c in range(nq):
                ps = psA.tile([128, 128], F32, tag="sc")
                nc.tensor.transpose(ps[:], qchunks[c // CH][:, c % CH, :], ident[:])
                nc.vector.tensor_scalar(
                    out=qnegT[:, c * 128:(c + 1) * 128], in0=ps[:],
                    scalar1=sign[:], scalar2=None, op0=MULT,
                )

            # Qswap^T = M' @ Qneg^T (M' undoes the sign and swaps even/odd partitions)
            qswapT = big.tile([128, sq], F32R, tag="qswapT")
            for n0 in range(0, sq, 512):
                ps = psB.tile([128, 512], F32, tag="pav")
                nc.tensor.matmul(
                    ps[:], swapneg[:],
                    qnegT[:, n0:n0 + 512],
                )
                nc.vector.tensor_copy(qswapT[:, n0:n0 + 512], ps[:])

            # V loads + per-chunk prep: V1 = fp32r-rounded copy (AV stationary),
            # V2 = [-vi, vr] interleaved. Chunked so DVE work stays fine-grained.
            vnat = big.tile([128, nk, 128], F32, tag="vnat")
            v1r = big.tile([128, nk, 128], BF16, tag="v1r")
            v2 = big.tile([128, nk, 128], BF16, tag="v2")
            vp = vnat.rearrange("p c (d two) -> p c d two", two=2)
            v2p = v2.rearrange("p c (d two) -> p c d two", two=2)
            for c0 in range(0, nk, CH):
                ce = min(c0 + CH, nk)
                nc.sync.dma_start(vnat[:, c0:ce, :], vv[:, c0:ce, :])
                nc.vector.tensor_copy(v1r[:, c0:ce, :], vnat[:, c0:ce, :])
                nc.vector.tensor_scalar(
                    out=v2p[:, c0:ce, :, 0], in0=vp[:, c0:ce, :, 1],
                    scalar1=-1.0, scalar2=None, op0=MULT,
                )
                nc.vector.tensor_copy(v2p[:, c0:ce, :, 1], vp[:, c0:ce, :, 0])

            def pe_consume(prev, comp, pav, psum, vsrc, pairs, quads, octs):
                """AV matmuls + denominator reduction for one exp'd group.

                Denominator: E slices pair-added then quad/oct-merged on DVE
                (bf16, 2x mode); one 128x512 ones-matmul per oct (8 k-tiles)
                streams through the PE.
                """
                et, g = prev
                for j in range(gk):
                    kt = g * gk + j
                    er = et[:, j * 512:(j + 1) * 512]
                    nc.tensor.matmul(
                        pav[:], vsrc[:, kt, :], er,
                        start=(kt == 0), stop=(kt == nk - 1),
                    )
                pr = small.tile([128, qb_size], BF16, tag=f"pair{comp}_{g % 3}")
                nc.vector.tensor_tensor(out=pr[:], in0=et[:, 0:512], in1=et[:, 512:1024], op=ADD)
                pairs.append(pr)
                oct_level = ngroups % 4 == 0 and ngroups >= 4
                if len(pairs) == 2:
                    qd = small.tile([128, qb_size], BF16, tag=f"quad{comp}_{(g // 2) % 2}")
                    nc.vector.tensor_tensor(out=qd[:], in0=pairs[0][:], in1=pairs[1][:], op=ADD)
                    pairs.clear()
                    if not oct_level:
                        h = g // 2
                        nc.tensor.matmul(
                            psum[:], onesm[:], qd[:],
                            start=(h == 0), stop=(h == ngroups // 2 - 1),
                        )
                    else:
                        quads.append(qd)
                        if len(quads) == 2:
                            oc = small.tile([128, qb_size], BF16, tag=f"oct{comp}")
                            nc.vector.tensor_tensor(out=oc[:], in0=quads[0][:], in1=quads[1][:], op=ADD)
                            quads.clear()
                            h = g // 4
                            nc.tensor.matmul(
                                psum[:], onesm[:], oc[:],
                                start=(h == 0), stop=(h == ngroups // 4 - 1),
                            )

            def make_qb_tail(qb, pavs, rhos):
                def run():
                    t0 = small.tile([128, qb_size], F32, tag="t0")
                    nc.vector.tensor_tensor(out=t0[:], in0=pavs[0][:], in1=rhos[0][:], op=MULT)
                    t1 = small.tile([128, qb_size], F32, tag="t1")
                    nc.vector.tensor_tensor(out=t1[:], in0=pavs[1][:], in1=rhos[1][:], op=MULT)
                    o = small.tile([128, qb_size], F32, tag="o")
                    nc.vector.tensor_tensor(out=o[:], in0=t0[:], in1=t1[:], op=ADD)

                    osb = small.tile([128, njb, 128], F32, tag="osb")
                    pt = psC.tile([128, 512], F32, tag="sum")
                    for j in range(njb):
                        nc.tensor.transpose(
                            pt[:, j * 128:(j + 1) * 128], o[:, j * 128:(j + 1) * 128],
                            ident[:],
                        )
                        nc.vector.tensor_copy(osb[:, j, :], pt[:, j * 128:(j + 1) * 128])
                    nc.sync.dma_start(ov[qb], osb[:])
                return run

            # Both complex components run as interleaved group streams: while
            # comp 0's exp is in flight on ACT, PE works comp 1's matmuls --
            # the exp handoff latency is fully hidden.
            rhs_srcs = (qnegT, qswapT)
            vsrcs = (v1r, v2)
            pending = None
            defer_g = min(2, ngroups - 1)
            for qb in range(nqb):
                pav = [psB.tile([128, qb_size], F32, tag="pav", name=f"pav{c}") for c in range(2)]
                psum = [psC.tile([128, qb_size], F32, tag="sum", name=f"sum{c}") for c in range(2)]
                prev = [None, None]
                pairs = [[], []]
                quads = [[], []]
                octs = [[], []]
                for g in range(ngroups):
                    for comp in range(2):
                        rhs_q = rhs_srcs[comp][:, qb * qb_size:(qb + 1) * qb_size]
                        sc = psA.tile([128, gk * 512], F32, tag="sc")
                        for j in range(gk):
                            kt = g * gk + j
                            nc.tensor.matmul(
                                sc[:, j * 512:(j + 1) * 512],
                                kT[:, kt * 128:(kt + 1) * 128],
                                rhs_q,
                            )
                        if prev[comp] is not None:
                            pe_consume(prev[comp], comp, pav[comp], psum[comp],
                                       vsrcs[comp], pairs[comp], quads[comp], octs[comp])
                        # previous q-block's combine/store runs here, hidden
                        # behind this block's early matmul stream
                        if pending is not None and comp == 0 and g == defer_g:
                            pending()
                            pending = None
                        et = epool.tile([128, gk * 512], BF16, tag=f"e{comp}")
                        nc.scalar.activation(et[:], sc[:], EXP, scale=SCALE)
                        prev[comp] = (et, g)
                rhos = []
                for comp in range(2):
                    pe_consume(prev[comp], comp, pav[comp], psum[comp],
                               vsrcs[comp], pairs[comp], quads[comp], octs[comp])
                    rho = small.tile([128, qb_size], F32, tag=f"rho{comp}")
                    nc.vector.reciprocal_approx_fast(rho[:], psum[comp][:])
                    rhos.append(rho)
                pending = make_qb_tail(qb, pav, rhos)
            pending()

    nc.compile()
    return nc


def host_consts():
    ident = np.eye(128, dtype=np.float32)
    # lhsT for Qswap^T = M' @ Qneg^T: lhsT[2d+1, 2d] = -1, lhsT[2d, 2d+1] = +1
    swapneg = np.zeros((128, 128), dtype=np.float32)
    idx = np.arange(0, 128, 2)
    swapneg[idx + 1, idx] = -1.0
    swapneg[idx, idx + 1] = 1.0
    import ml_dtypes
    onesm = np.ones((128, 128), dtype=ml_dtypes.bfloat16)
    sign = np.empty((128, 1), dtype=np.float32)
    sign[0::2] = 1.0
    sign[1::2] = -1.0
    return {"ident": ident, "swapneg": swapneg, "onesm": onesm, "sign": sign}


_LAST_RESULTS = [None]  # BassKernelResults stash for test harness introspection


def kernel(queries, keys, values):
    from concourse.bass_utils import run_bass_kernel_spmd

    queries = np.ascontiguousarray(np.asarray(queries, dtype=np.float32))
    keys = np.ascontiguousarray(np.asarray(keys, dtype=np.float32))
    values = np.ascontiguousarray(np.asarray(values, dtype=np.float32))
    assert queries.shape == (B, S, D, 2), queries.shape

    nc = build_nc()
    consts = host_consts()
    halves = S // (NCORES // B)  # 2048 rows per core
    in_maps = []
    for c in range(NCORES):
        b, h = c // 2, c % 2
        in_maps.append({
            "q": queries[b, h * halves:(h + 1) * halves].reshape(SQ, W),
            "k": keys[b].reshape(S, W),
            "v": values[b].reshape(S, W),
            **consts,
        })
    res = run_bass_kernel_spmd(
        nc, in_maps, list(range(NCORES)),
        trace=bool(int(os.environ.get("KERNEL_TRACE", "0"))),
    )
    _LAST_RESULTS[0] = res
    out = np.empty((B, S, D, 2), dtype=np.float32)
    for c in range(NCORES):
        b, h = c // 2, c % 2
        out[b, h * halves:(h + 1) * halves] = res.results[c]["out"].reshape(halves, D, 2)
    return out

